# revision 1
# baseline (speedup 1.0000x reference)
"""Trainium2 Bass kernel for nn_DarkCLoss: loss = -mean(|maxpool3d_{3,35,35}(1-x)|).

Math: with p=35 and -inf padding, the reference is
    loss = -mean(1 - minpool2d_35x35(min_c x)) = mean(minpool) - 1
so we compute the 2D sliding-window min (window 35, stride 1, +inf pads)
of the channel-min, sum it, and finish on the host.

Sharding: pure data-parallel, 2 images per core across 8 cores; each core
returns its partial sum of the pooled map; host combines (the all-reduce
of the scalar loss from the sharding hint, done on host).

Device algorithm per image (all pooling exact in bf16; inputs are shipped
as bf16 — the pooled term is ~2.7e-4 of the loss, so bf16 rounding of the
input perturbs the result by ~1e-6 relative):
  - 3 channel DMAs into [128, 4, 512] tiles (row h = 128*hc + p).
  - channel min: 2 DVE tensor_tensor ops; the second writes into the
    +inf-padded W buffer [128, 4, 560] (pad 18 left / 30 right).
  - sliding-min-35 along W via a log2 doubling chain of shifted
    tensor_tensor mins (shifts 1,2,4,8,16,3).  Odd shifts are made
    4-byte aligned by materializing the shifted operand with a GPSIMD
    copy, so every DVE op runs in the 2x bf16 perf mode.
  - PE transposes 16 [128,128] blocks into PSUM; ScalarE copies PSUM into
    the padded H buffer; same doubling chain along H.
  - PE ones-matmul accumulates the partition sums of the pooled map into
    one PSUM bank across both images; one DVE reduce drains it to a
    scalar that is DMA'd out.
"""

import numpy as np
import ml_dtypes

import concourse.bacc as bacc
import concourse.tile as tile
import concourse.mybir as mybir
from concourse.alu_op_type import AluOpType
from concourse.bass_utils import run_bass_kernel_spmd
from concourse.masks import make_identity

N_CORES = 8
B, C, H, W = 16, 3, 512, 512
B_LOC = B // N_CORES          # images per core
K = 35                        # pool window
PAD_L = 18                    # left pad (data starts 4B-aligned)
SEG = 560                     # padded segment width (>= 512 + 18 + 30)
HC = 4                        # 512 rows = 4 blocks of 128 partitions
INF = float("inf")

_CACHE = {}


def _chain(nc, pool, buf, img_tag):
    """Sliding-min-35 along the innermost dim of padded buf [128, 4, SEG].

    Valid window results land in the returned [128, 4, 512] tile:
    out[:, s, h] = min(data[h-17 .. h+17]) of segment s (inf-padded).
    Data lives at buf[:, :, PAD_L : PAD_L+512]; in padded coords the
    window starting at data index h-17 begins at column j = h+1.
    """
    bf16 = mybir.dt.bfloat16
    mn = AluOpType.min
    # shift 1 via GPSIMD shadow copy, then 2x-mode TT
    sh1 = pool.tile([128, HC, 546], bf16, tag=f"sh1{img_tag}")
    nc.gpsimd.tensor_copy(out=sh1, in_=buf[:, :, 1:547])
    m2 = pool.tile([128, HC, 546], bf16, tag=f"cha{img_tag}")
    nc.vector.tensor_tensor(out=m2, in0=buf[:, :, 0:546], in1=sh1, op=mn)
    # doublings 2, 4, 8 ping-pong
    m4 = pool.tile([128, HC, 544], bf16, tag=f"chb{img_tag}")
    nc.vector.tensor_tensor(out=m4, in0=m2[:, :, 0:544], in1=m2[:, :, 2:546], op=mn)
    m8 = pool.tile([128, HC, 540], bf16, tag=f"chc{img_tag}")
    nc.vector.tensor_tensor(out=m8, in0=m4[:, :, 0:540], in1=m4[:, :, 4:544], op=mn)
    m16 = pool.tile([128, HC, 532], bf16, tag=f"chd{img_tag}")
    nc.vector.tensor_tensor(out=m16, in0=m8[:, :, 0:532], in1=m8[:, :, 8:540], op=mn)
    m32 = pool.tile([128, HC, 516], bf16, tag=f"che{img_tag}")
    nc.vector.tensor_tensor(out=m32, in0=m16[:, :, 0:516], in1=m16[:, :, 16:532], op=mn)
    # final: out[j=1..512] = min(M32[j], M32[j+3]); odd operand via GPSIMD
    sh3 = pool.tile([128, HC, 512], bf16, tag=f"sh3{img_tag}")
    nc.gpsimd.tensor_copy(out=sh3, in_=m32[:, :, 1:513])
    out = pool.tile([128, HC, 512], bf16, tag=f"out{img_tag}")
    nc.vector.tensor_tensor(out=out, in0=sh3, in1=m32[:, :, 4:516], op=mn)
    return out


def _build():
    if "nc" in _CACHE:
        return _CACHE["nc"]
    bf16 = mybir.dt.bfloat16
    f32 = mybir.dt.float32
    mn = AluOpType.min

    nc = bacc.Bacc("TRN2", target_bir_lowering=False, debug=False)
    x = nc.dram_tensor("x", [B_LOC, C, H, W], bf16, kind="ExternalInput")
    out_d = nc.dram_tensor("out", [1, 1], f32, kind="ExternalOutput")

    with tile.TileContext(nc) as tc:
        with (
            tc.tile_pool(name="consts", bufs=1) as consts,
            tc.tile_pool(name="work", bufs=2) as work,
            tc.tile_pool(name="pswork", bufs=2, space="PSUM") as pswork,
            tc.tile_pool(name="psacc", bufs=1, space="PSUM") as psacc,
        ):
            ident = consts.tile([128, 128], bf16)
            make_identity(nc, ident)
            ones = consts.tile([128, 1], bf16)
            nc.vector.memset(ones, 1.0)
            acc = psacc.tile([1, 512], f32)

            for b in range(B_LOC):
                # --- load channels (h = 128*hc + p) -------------------
                ct = []
                for c in range(C):
                    t = work.tile([128, HC, 512], bf16, tag=f"c{c}")
                    src = x[b, c].rearrange("(hc p) w -> p hc w", p=128)
                    nc.sync.dma_start(out=t, in_=src)
                    ct.append(t)
                # --- channel min into padded W buffer -----------------
                t1 = work.tile([128, HC, 512], bf16, tag="t1")
                nc.vector.tensor_tensor(out=t1, in0=ct[0], in1=ct[1], op=mn)
                wbuf = work.tile([128, HC, SEG], bf16, tag="wbuf")
                nc.gpsimd.memset(wbuf[:, :, 0:PAD_L], INF)
                nc.gpsimd.memset(wbuf[:, :, PAD_L + 512:SEG], INF)
                nc.vector.tensor_tensor(
                    out=wbuf[:, :, PAD_L:PAD_L + 512], in0=t1, in1=ct[2], op=mn)
                # --- sliding min along W ------------------------------
                wmin = _chain(nc, work, wbuf, "w")
                # --- transpose to [w-part, h-free] --------------------
                pt = pswork.tile([128, HC, 512], bf16)
                for k in range(HC):
                    for hc in range(HC):
                        nc.tensor.transpose(
                            pt[:, k, 128 * hc:128 * (hc + 1)],
                            wmin[:, hc, 128 * k:128 * (k + 1)],
                            ident)
                hbuf = work.tile([128, HC, SEG], bf16, tag="hbuf")
                nc.gpsimd.memset(hbuf[:, :, 0:PAD_L], INF)
                nc.gpsimd.memset(hbuf[:, :, PAD_L + 512:SEG], INF)
                nc.scalar.copy(out=hbuf[:, :, PAD_L:PAD_L + 512], in_=pt)
                # --- sliding min along H ------------------------------
                hmin = _chain(nc, work, hbuf, "h")
                # --- accumulate partition sums on PE ------------------
                for k in range(HC):
                    nc.tensor.matmul(
                        acc[0:1, :], ones, hmin[:, k, :],
                        start=(b == 0 and k == 0),
                        stop=(b == B_LOC - 1 and k == HC - 1))

            total = consts.tile([1, 1], f32)
            nc.vector.reduce_sum(
                out=total, in_=acc[0:1, :], axis=mybir.AxisListType.X)
            nc.sync.dma_start(out=out_d[:, :], in_=total)

    nc.compile()
    _CACHE["nc"] = nc
    return nc


def run(x, trace=False):
    """x: [16,3,512,512] float32. Returns (loss_scalar, exec_time_ns)."""
    nc = _build()
    xb = np.ascontiguousarray(x).astype(ml_dtypes.bfloat16)
    in_maps = [
        {"x": np.ascontiguousarray(xb[i * B_LOC:(i + 1) * B_LOC])}
        for i in range(N_CORES)
    ]
    res = run_bass_kernel_spmd(
        nc, in_maps, core_ids=list(range(N_CORES)), trace=trace)
    total = sum(float(r["out"][0, 0]) for r in res.results)
    loss = total / float(B * H * W) - 1.0
    return np.float32(loss), res.exec_time_ns


def kernel(x):
    loss, _ = run(x)
    return loss


# revision 3
# speedup vs baseline: 1.6987x; 1.6987x over previous
"""Trainium2 Bass kernel for nn_DarkCLoss: loss = -mean(|maxpool3d_{3,35,35}(1-x)|).

Math: with p=35 and -inf padding, the reference is
    loss = -mean(1 - minpool2d_35x35(min_c x)) = mean(minpool) - 1
so we compute the 2D sliding-window min (window 35, stride 1, +inf pads)
of the channel-min, sum it, and finish on the host.

Sharding: pure data-parallel, 2 images per core across 8 cores; each core
returns its partial sum of the pooled map; host combines (the scalar
all-reduce from the sharding hint, done on host).

Device algorithm per image (all pooling exact in bf16; inputs shipped as
bf16 — the pooled term is ~2.7e-4 of the loss, so bf16 rounding of the
input perturbs the result by ~1e-6 relative):
  - 3 channel DMAs land in the interiors of +inf-padded [128, 4, 560]
    tiles (row h = 128*hc + p; 4 row-blocks as padded segments side by
    side in the free dim).
  - channel min: 2 DVE tensor_tensor ops on FLAT [128, 2240] views (flat
    2D APs keep the DVE in its 2x bf16 perf mode; segment-crossing reads
    only pollute positions no valid output depends on, because every
    valid 35-window's dependency cone stays inside one padded segment).
  - sliding-min-35 along W via a log2 doubling chain of shifted flat
    tensor_tensor mins (shifts 1,2,4,8,16,3).  Odd shifts are made
    4-byte aligned by materializing the shifted operand with a ScalarE
    copy, so every DVE op stays in 2x mode.
  - PE transposes 16 [128,128] blocks into PSUM; ScalarE copies PSUM into
    the padded H buffer; same doubling chain along H.
  - PE ones-matmul accumulates the partition sums of the pooled map into
    one PSUM bank across both images; one DVE reduce drains it to a
    scalar that is DMA'd out.
"""

import numpy as np
import ml_dtypes

import concourse.bacc as bacc
import concourse.tile as tile
import concourse.mybir as mybir
from concourse.alu_op_type import AluOpType
from concourse.bass_utils import run_bass_kernel_spmd
from concourse.masks import make_identity

N_CORES = 8
B, C, H, W = 16, 3, 512, 512
B_LOC = B // N_CORES          # images per core
K = 35                        # pool window
PAD_L = 18                    # left pad (data starts 4B-aligned)
SEG = 560                     # padded segment width (>= 512 + 18 + 30)
HC = 4                        # 512 rows = 4 blocks of 128 partitions
FLAT = HC * SEG               # 2240
INF = float("inf")

_CACHE = {}


def _chain(nc, pool, buf, img_tag):
    """Sliding-min-35 along flat padded buf [128, FLAT] (bf16, inf pads).

    Returns a flat [128, FLAT] tile whose columns SEG*s + (1..512) hold
    the valid window mins of segment s: out[SEG*s + 1 + h] =
    min(data_s[h-17 .. h+17]).  All DVE ops are flat 2D and 4B-aligned
    (odd shifts go through a ScalarE shadow copy), so they run at the
    2x bf16 rate.  Garbage outside the valid columns is never read.
    """
    bf16 = mybir.dt.bfloat16
    mn = AluOpType.min
    fl = lambda t: t.rearrange("p a b -> p (a b)") if len(t.shape) == 3 else t
    b2 = fl(buf)

    def tl(tag):
        return pool.tile(
            [128, FLAT], bf16, name=f"{tag}{img_tag}", tag=f"{tag}{img_tag}")

    sh1 = tl("sh1")
    nc.scalar.copy(out=sh1[:, 0:FLAT - 2], in_=b2[:, 1:FLAT - 1])
    m2 = tl("cha")
    nc.vector.tensor_tensor(
        out=m2[:, 0:2238], in0=b2[:, 0:2238], in1=sh1[:, 0:2238], op=mn)
    m4 = tl("chb")
    nc.vector.tensor_tensor(
        out=m4[:, 0:2236], in0=m2[:, 0:2236], in1=m2[:, 2:2238], op=mn)
    m8 = tl("chc")
    nc.vector.tensor_tensor(
        out=m8[:, 0:2232], in0=m4[:, 0:2232], in1=m4[:, 4:2236], op=mn)
    m16 = tl("chd")
    nc.vector.tensor_tensor(
        out=m16[:, 0:2224], in0=m8[:, 0:2224], in1=m8[:, 8:2232], op=mn)
    m32 = tl("che")
    nc.vector.tensor_tensor(
        out=m32[:, 0:2208], in0=m16[:, 0:2208], in1=m16[:, 16:2224], op=mn)
    # final: out[j] = min(M32[j], M32[j+3]); the +3 operand via ScalarE
    sh3 = tl("sh3")
    nc.scalar.copy(out=sh3[:, 0:2204], in_=m32[:, 3:2207])
    out = tl("out")
    nc.vector.tensor_tensor(
        out=out[:, 0:2204], in0=m32[:, 0:2204], in1=sh3[:, 0:2204], op=mn)
    return out


def _build():
    if "nc" in _CACHE:
        return _CACHE["nc"]
    bf16 = mybir.dt.bfloat16
    f32 = mybir.dt.float32
    mn = AluOpType.min

    nc = bacc.Bacc("TRN2", target_bir_lowering=False, debug=False)
    x = nc.dram_tensor("x", [B_LOC, C, H, W], bf16, kind="ExternalInput")
    out_d = nc.dram_tensor("out", [1, 1], f32, kind="ExternalOutput")

    with tile.TileContext(nc) as tc:
        with (
            tc.tile_pool(name="consts", bufs=1) as consts,
            tc.tile_pool(name="work", bufs=2) as work,
            tc.tile_pool(name="pswork", bufs=2, space="PSUM") as pswork,
            tc.tile_pool(name="psacc", bufs=1, space="PSUM") as psacc,
        ):
            ident = consts.tile([128, 128], bf16)
            make_identity(nc, ident)
            ones = consts.tile([128, 1], bf16)
            nc.vector.memset(ones, 1.0)
            acc = psacc.tile([1, 512], f32)

            for b in range(B_LOC):
                # --- load channels into padded tiles (h = 128*hc + p) --
                ct = []
                for c in range(C):
                    t = work.tile([128, HC, SEG], bf16, tag=f"c{c}")
                    src = x[b, c].rearrange("(hc p) w -> p hc w", p=128)
                    nc.sync.dma_start(out=t[:, :, PAD_L:PAD_L + 512], in_=src)
                    ct.append(t)
                cf = [t.rearrange("p a b -> p (a b)") for t in ct]
                # --- channel min (flat), then fix pads to +inf --------
                t1 = work.tile([128, FLAT], bf16, tag="t1")
                nc.vector.tensor_tensor(out=t1, in0=cf[0], in1=cf[1], op=mn)
                wbuf = work.tile([128, HC, SEG], bf16, tag="wbuf")
                nc.vector.tensor_tensor(
                    out=wbuf.rearrange("p a b -> p (a b)"), in0=t1, in1=cf[2],
                    op=mn)
                nc.gpsimd.memset(wbuf[:, :, 0:PAD_L], INF)
                nc.gpsimd.memset(wbuf[:, :, PAD_L + 512:SEG], INF)
                # --- sliding min along W ------------------------------
                wmin = _chain(nc, work, wbuf, "w")
                # --- transpose to [w-part, h-free] --------------------
                pt = pswork.tile([128, HC, 512], bf16)
                for k in range(HC):
                    for hc in range(HC):
                        nc.tensor.transpose(
                            pt[:, k, 128 * hc:128 * (hc + 1)],
                            wmin[:, SEG * hc + 1 + 128 * k:
                                 SEG * hc + 1 + 128 * (k + 1)],
                            ident)
                hbuf = work.tile([128, HC, SEG], bf16, tag="hbuf")
                nc.gpsimd.memset(hbuf[:, :, 0:PAD_L], INF)
                nc.gpsimd.memset(hbuf[:, :, PAD_L + 512:SEG], INF)
                nc.scalar.copy(out=hbuf[:, :, PAD_L:PAD_L + 512], in_=pt)
                # --- sliding min along H ------------------------------
                hmin = _chain(nc, work, hbuf, "h")
                # --- accumulate partition sums on PE ------------------
                for k in range(HC):
                    nc.tensor.matmul(
                        acc[0:1, :], ones,
                        hmin[:, SEG * k + 1:SEG * k + 513],
                        start=(b == 0 and k == 0),
                        stop=(b == B_LOC - 1 and k == HC - 1))

            total = consts.tile([1, 1], f32)
            nc.vector.reduce_sum(
                out=total, in_=acc[0:1, :], axis=mybir.AxisListType.X)
            nc.sync.dma_start(out=out_d[:, :], in_=total)

    nc.compile()
    _CACHE["nc"] = nc
    return nc


def run(x, trace=False):
    """x: [16,3,512,512] float32. Returns (loss_scalar, exec_time_ns)."""
    nc = _build()
    xb = np.ascontiguousarray(x).astype(ml_dtypes.bfloat16)
    in_maps = [
        {"x": np.ascontiguousarray(xb[i * B_LOC:(i + 1) * B_LOC])}
        for i in range(N_CORES)
    ]
    res = run_bass_kernel_spmd(
        nc, in_maps, core_ids=list(range(N_CORES)), trace=trace)
    total = sum(float(r["out"][0, 0]) for r in res.results)
    loss = total / float(B * H * W) - 1.0
    return np.float32(loss), res.exec_time_ns


def kernel(x):
    loss, _ = run(x)
    return loss
